# revision 66
# baseline (speedup 1.0000x reference)
"""Multi-head attention (B=4, S=2048, D=768, H=12) on 8 trn2 NeuronCores.

Sharding: core c handles batch b = c//2 and head-half hh = c%2 (6 heads,
384 features). Each core computes a partial output [2048, 768] (its 6 heads'
contribution through the output projection, un-biased); the host sums the
two partials per batch and adds OW_b plus the V-bias constant
(softmax rows sum to 1, so the V bias contributes OW_w @ VW_b per token).

Speed structure (cost-model-guided; exp throughput is the wall):
  x / weights arrive pre-transposed AND pre-cast to bf16 from the host.
  Q/K projections write fp8e4 tiles (bias fused in the evac) in a "folded"
  layout: [128, 2*S] per m-chunk, col-block 1 = feats 32..63 folded down to
  partitions 0..31 / 64..95 by sbuf DMAs, making the 64-deep score
  contraction 2 k-tiles of 32 partitions -> one fp8 DoubleRow matmul per
  (kc, qb) at 0.5 cycles/row.
  exp is split tile-alternating between ScalarE (exact exp -> fp8,
  scale fused) and DVE (Schraudolph: fp8 bits = round(s*A + B) as a
  single tensor_scalar mult+add writing uint8, bit-viewed as fp8e4;
  its log-domain quantization ~ coincides with fp8's own mantissa
  rounding).
  AV: stationary-P fp8 DoubleRow over k-chunk pairs: out[128q, 65] +=
  P^T-pair.T @ [V8 | ones]-pair plus a residual pass against [R8 | zeros]
  (R8 = fp8(V - fp8(V)), built on the otherwise-idle GpSimd engine in
  per-512-token quarters pipelined behind the V evacs) -- V stays
  ~bf16-accurate with both operands fp8.  The ones column drops the
  softmax denominator at column 64 of each qtile block; normalize fuses
  into the psum evac as one broadcast tensor_tensor against the
  reciprocal column.
  attn_sb [128q, qt*128f] xbar-transposes (1 DMA per (qb, m, pa-half))
  into attnT [128f, tok] for the output projection.

Scheduling: units run qb-major ((qb, m, hh)) so all qb=0 attention
finishes by unit 5 and the first 8 output tiles overlap the whole qb=1
phase; Q-projection halves for qb=1 are deferred into the qb=1 phase.
Every engine queue is strictly FIFO (no bypass), so emission order is a
schedule: cold work carries hard (unit, kc) deadlines in a heap and is
fenced before each score tile, producer->consumer chains are spread
3-4 slots apart so semaphore waits never sit at a queue head, and AV /
normalize / transpose tasks pop from kc slot 4 so their inputs are
complete by emission.  exp is ACT-exact vs DVE-Schraudolph, 10.2:5.8
in the PE-bound qb=0 phase and 9.58:6.42 in the exp-bound qb=1 phase
(GPSIMD cannot read PSUM, so only these two can consume score tiles;
Pool gets all SBUF-only work, and V/proj/out evacs sit on DVE because
any fixed work on ACT stretches the exp wall).  The output writes back
as f16 to halve the final DMA flush.
PSUM: 3 score slots [128,1024] (6 banks) + a 2-deep aux ring [128,512]
(2 banks) shared by projection chunks and AV accumulation, so AV pa1
never serializes against normalize(pa0).
"""

import heapq
from collections import deque

import numpy as np
import ml_dtypes

import concourse.bass as bass
import concourse.mybir as mybir
import concourse.tile as tile
from concourse.bass_utils import run_bass_kernel_spmd

F32 = mybir.dt.float32
BF16 = mybir.dt.bfloat16
FP8 = mybir.dt.float8e4
U8 = mybir.dt.uint8
AF = mybir.ActivationFunctionType
ALU = mybir.AluOpType
DR = mybir.MatmulPerfMode.DoubleRow

B, S, D = 4, 2048, 768
H, HD = 12, 64
N_CORES = 8
HEADS_PER_CORE = 6
FS = HEADS_PER_CORE * HD    # 384
KT16 = S // 128             # 16
QB = 1024
SCALE = 0.125               # 1/sqrt(64)

# Schraudolph exp: fp8e4 bits = round(s_raw * SCH_A + SCH_B).
# SCH_A = 0.125 (score scale) * 8 / ln 2; SCH_B tuned for min final error
# (56 - 0.46; hardware f32->u8 conversion rounds to nearest).
SCH_A = 1.4426950408889634
SCH_B = 55.54
# exp engine weights per 16 kc tiles: ACT exact exp (1038ns/tile) vs
# DVE Schraudolph (1192ns/tile + its evac/norm work).  GPSIMD cannot
# read PSUM on TRN2, so the scores tiles can only be consumed by these
# two; Pool takes the SBUF-only work (v-path quantize, memsets) instead.
EXP_W0 = {"act": 10.2, "dve": 5.8}
EXP_W1 = {"act": 9.58, "dve": 6.42}


def split_waits(nc, cap=1):
    """walrus rejects instructions carrying >2 sync waits; the TileContext
    final drain is emitted post-lowering and can carry many. Hoist excess
    waits onto preceding same-engine NOPs (1 wait each)."""
    f = nc.m.functions[0]
    for bb in f.blocks:
        insts = list(bb.instructions)
        new = []
        changed = False
        for inst in insts:
            si = inst.sync_info
            if si is not None and si.on_wait is not None and len(si.on_wait) > cap:
                waits = list(si.on_wait)
                keep = waits[-cap:]
                extra = waits[:-cap]
                for j, w in enumerate(extra):
                    nop = mybir.InstNoOp(
                        name=f"{inst.name}-wsplit{j}",
                        engine=inst.engine,
                        ins=[], outs=[],
                        sync_info=mybir.SyncInfo(on_wait=[w], on_update=[]),
                    )
                    new.append(nop)
                    changed = True
                inst.sync_info = mybir.SyncInfo(
                    on_wait=keep, on_update=list(si.on_update or [])
                )
            new.append(inst)
        if changed:
            bb.instructions = new


def build_nc(reps=1):
    nc = bass.Bass()

    x_ext = nc.dram_tensor("xt", [D, S], BF16, kind="ExternalInput")
    wq_ext = nc.dram_tensor("wqt", [D, FS], BF16, kind="ExternalInput")
    wk_ext = nc.dram_tensor("wkt", [D, FS], BF16, kind="ExternalInput")
    wv_ext = nc.dram_tensor("wvt", [D, FS], BF16, kind="ExternalInput")
    wo_ext = nc.dram_tensor("wot", [FS, D], BF16, kind="ExternalInput")
    bq_ext = nc.dram_tensor("bq", [FS], F32, kind="ExternalInput")
    bk_ext = nc.dram_tensor("bk", [FS], F32, kind="ExternalInput")
    id_ext = nc.dram_tensor("ident", [128, 128], BF16, kind="ExternalInput")
    # f16 halves the serial-DMA cost of the 6MB output writeback; the
    # host upcasts when summing partials (f16 rounding is ~0.05% rel,
    # invisible next to the fp8 score/exp quantization)
    out_ext = nc.dram_tensor("out_part", [S, D], mybir.dt.float16,
                             kind="ExternalOutput")

    with tile.TileContext(nc) as tc:
      for _rep in range(reps):
        with tc.tile_pool(name="persist", bufs=1) as P:
            xT = P.tile([128, 6 * S], BF16, name="xT")
            xTv = xT[:].rearrange("p (j s) -> p j s", j=6)
            wT = {p: P.tile([128, 6 * FS], BF16, name=f"wT{p}") for p in "qkv"}
            woT = P.tile([128, 3 * D], BF16, name="woT")
            QT8 = [P.tile([128, 2 * S], FP8, name=f"QT8{m}") for m in range(3)]
            KT8 = [P.tile([128, 2 * S], FP8, name=f"KT8{m}") for m in range(3)]
            VT = [P.tile([128, S], BF16, name=f"VT{m}") for m in range(3)]
            vtf = [P.tile([128, KT16 * 64], BF16, name=f"vtf{h}")
                   for h in range(HEADS_PER_CORE)]
            v18 = [P.tile([128, KT16 * 65], FP8, name=f"v18h{h}")
                   for h in range(HEADS_PER_CORE)]
            r8 = [P.tile([128, KT16 * 65], FP8, name=f"r8h{h}")
                  for h in range(HEADS_PER_CORE)]
            Pt = [P.tile([128, KT16 * QB], FP8, name=f"Pt{i}")
                  for i in range(2)]
            attn_sb = {(qb, m): P.tile([128, 1024], BF16, name=f"asb{qb}_{m}")
                       for qb in range(2) for m in range(3)}
            attnT = [P.tile([128, S], BF16, name=f"attnT{m}") for m in range(3)]
            qb_sb = P.tile([128, 3], F32, name="qb_sb")
            kb_sb = P.tile([128, 3], F32, name="kb_sb")
            ident = P.tile([128, 128], BF16, name="ident")

            warm = P.tile([128, 1], F32, name="warm")
            nc.vector.memset(warm[:], 0.0)
            nc.scalar.activation(warm[:], warm[:], AF.Exp)

            with (
                tc.tile_pool(name="sp", bufs=1, space="PSUM") as SP,
                tc.tile_pool(name="nw", bufs=3) as NW,
                tc.tile_pool(name="pw", bufs=3) as PW,
            ):
                # ---------- loads (host pre-transposed, pre-cast bf16).
                # x arrives in quarter/half chunks so the m=0 K projection
                # starts after ~1/4 of the x traffic; early chunks on the
                # sync queue, tail half on the idle vector queue.  Weights
                # ride the scalar queue (Activation idle at start). -------
                xe = x_ext.rearrange("(j p) s -> p j s", p=128)
                # first quarter in two halves: the K s4-0 matmuls for
                # d-chunks 0-2 start while chunks 3-5 are still in flight
                nc.sync.dma_start(xTv[:, 0:3, 0:512], xe[:, 0:3, 0:512])
                nc.sync.dma_start(xTv[:, 3:6, 0:512], xe[:, 3:6, 0:512])
                for wtile, wext in ((wT["k"], wk_ext), (wT["q"], wq_ext)):
                    nc.scalar.dma_start(
                        wtile[:].rearrange("p (j f) -> p j f", j=6)[:, :, 0:128],
                        wext.rearrange("(j p) f -> p j f", p=128)[:, :, 0:128])
                nc.sync.dma_start(xTv[:, :, 512:QB], xe[:, :, 512:QB])

                def load_w_rest(p):
                    def run(p=p):
                        ext = wq_ext if p == "q" else wk_ext
                        nc.scalar.dma_start(
                            wT[p][:].rearrange(
                                "p (j f) -> p j f", j=6)[:, :, 128:FS],
                            ext.rearrange(
                                "(j p) f -> p j f", p=128)[:, :, 128:FS])
                    return run
                nc.scalar.dma_start(ident[:], id_ext[:, :])
                nc.scalar.dma_start(qb_sb[:],
                                    bq_ext.rearrange("(j p) -> p j", p=128))
                nc.scalar.dma_start(kb_sb[:],
                                    bk_ext.rearrange("(j p) -> p j", p=128))
                nc.sync.dma_start(xTv[:, :, QB:S], xe[:, :, QB:S])
                nc.scalar.dma_start(
                    wT["v"][:].rearrange("p (j f) -> p j f", j=6),
                    wv_ext.rearrange("(j p) f -> p j f", p=128))

                def load_wo():
                    nc.scalar.dma_start(
                        woT[:].rearrange("p (j f) -> p j f", j=3),
                        wo_ext.rearrange("(j p) f -> p j f", p=128))

                # ---------- psum slot managers ----------------------------
                sc_idx = [0]

                def sc_tile():
                    i = sc_idx[0] % 3
                    sc_idx[0] += 1
                    return SP.tile([128, QB], F32, tag=f"s{i}", bufs=1,
                                   name=f"ps{i}")

                def aux_tile():
                    return SP.tile([128, 512], F32, tag="aux", bufs=2,
                                   name="aux")

                # ---------- exp engine interleave (error diffusion) -------
                exp_acc = {"act": 0.0, "dve": 0.0}

                def exp_engine(w):
                    for e, wt in w.items():
                        exp_acc[e] += wt
                    e = max(exp_acc, key=exp_acc.get)
                    exp_acc[e] -= sum(w.values())
                    return e

                # ---------- emission helpers ------------------------------
                def proj_chunks(p, m, s4, tile_fn=None):
                    """projection p, m-chunk, quarter s4 -> two tasks:
                    [alloc+6mm], [evac] (evac decoupled so it never
                    head-blocks the in-order exp engines)"""
                    st = {}

                    def c0():
                        st["ps"] = (tile_fn or aux_tile)()
                        for kc in range(6):
                            nc.tensor.matmul(
                                st["ps"][:, 0:512],
                                wT[p][:, kc * FS + m * 128:
                                      kc * FS + (m + 1) * 128],
                                xTv[:, kc, s4 * 512:(s4 + 1) * 512],
                                start=(kc == 0), stop=(kc == 5),
                            )

                    def c1():
                        cols = slice(s4 * 512, (s4 + 1) * 512)
                        ps = st["ps"][:, 0:512]
                        if p == "q":
                            nc.vector.tensor_scalar_add(
                                QT8[m][:, cols], ps, qb_sb[:, m:m + 1])
                        elif p == "k":
                            nc.vector.tensor_scalar_add(
                                KT8[m][:, cols], ps, kb_sb[:, m:m + 1])
                        else:
                            nc.vector.tensor_copy(VT[m][:, cols], ps)

                    return c0, c1

                def fold(t8, c0, c1):
                    # sync queue: a fold's wait on its evac may block the
                    # queue head, and SP has nothing urgent to block
                    nc.sync.dma_start(t8[0:32, S + c0:S + c1],
                                      t8[32:64, c0:c1])
                    nc.sync.dma_start(t8[64:96, S + c0:S + c1],
                                      t8[96:128, c0:c1])

                def vpath_quarter(h, s4):
                    """transpose one 512-token quarter of V head h into
                    [tok, feat] and build the fp8 main + residual AV
                    operands for it.  SBUF-only, so the copy/subtract run
                    on the otherwise-idle Pool engine; quartering caps the
                    latency from last V-evac to AV-ready at ~1.7us."""
                    m, hh = h // 2, h % 2
                    po = hh * 64
                    ts = slice(4 * s4, 4 * s4 + 4)
                    v1v = v18[h][:].rearrange("p (t c) -> p t c", t=KT16)
                    r1v = r8[h][:].rearrange("p (t c) -> p t c", t=KT16)
                    vtv = vtf[h][:].rearrange("p (t c) -> p t c", t=KT16)
                    tasks = [
                        lambda: nc.sync.dma_start_transpose(
                            vtv[:, ts, :],
                            VT[m][po:po + 64, s4 * 512:(s4 + 1) * 512]),
                        lambda: nc.gpsimd.tensor_copy(
                            v1v[:, ts, 0:64], vtv[:, ts, :]),
                        lambda: nc.gpsimd.tensor_tensor(
                            r1v[:, ts, 0:64], vtv[:, ts, :],
                            v1v[:, ts, 0:64], ALU.subtract),
                    ]
                    if s4 == 0:
                        ms = [lambda: (nc.gpsimd.memset(v1v[:, :, 64:65], 1.0),
                                       nc.gpsimd.memset(r1v[:, :, 64:65], 0.0))]
                        return ms + tasks
                    return tasks

                def out_proj_task(t, eng):
                    def run():
                        ot = PW.tile([128, D], mybir.dt.float16, tag="ot",
                                     name="ot", bufs=8)
                        ps = sc_tile()
                        for (lo, hi) in ((0, 512), (512, 768)):
                            for mc in range(3):
                                nc.tensor.matmul(
                                    ps[:, lo:hi],
                                    attnT[mc][:, t * 128:(t + 1) * 128],
                                    woT[:, mc * D + lo:mc * D + hi],
                                    start=(mc == 0), stop=(mc == 2),
                                )
                        if eng == "act":
                            nc.scalar.activation(ot[:], ps[:, 0:D], AF.Copy)
                        elif eng == "dve":
                            nc.vector.tensor_copy(ot[:], ps[:, 0:D])
                        else:
                            nc.gpsimd.tensor_copy(ot[:], ps[:, 0:D])
                        nc.sync.dma_start(
                            out_ext[t * 128:(t + 1) * 128, :], ot[:])
                    return run

                hot = deque()
                cold = []  # heap of (due_ui, due_kc, seq, fn)
                cseq = [0]

                def queue_at(ui, kc, fn):
                    heapq.heappush(cold, (ui, kc, cseq[0], fn))
                    cseq[0] += 1

                def fence(ui, kc):
                    # mandatory: everything due at or before (ui, kc) must
                    # be emitted before slot kc's score matmul, so the
                    # dependency graph (defined by emission order) is right
                    # and work streams instead of bursting at unit edges
                    while cold and (cold[0][0], cold[0][1]) <= (ui, kc):
                        heapq.heappop(cold)[3]()

                def queue_mchunk(mm):
                    # projections + v-path for m-chunk mm, first used by
                    # unit u2 = 2*mm (qb=0 phase); chains are spread so no
                    # engine sees a burst and every K fold meets its kc
                    # deadline (fold s4-j needed before kc 4j)
                    p, u2 = 2 * mm - 1, 2 * mm
                    for s4, (du, dk) in enumerate(((p, 0), (p, 4))):
                        c0, c1 = proj_chunks("q", mm, s4)
                        queue_at(du, dk, c0)
                        queue_at(du, dk + 2, c1)
                    queue_at(p, 7, lambda: fold(QT8[mm], 0, QB))
                    kd = [(p, 8), (p, 10), (p, 11),
                          (p, 12), (p, 14), (p, 15),
                          (u2, 0), (u2, 2), (u2, 3),
                          (u2, 4), (u2, 6), (u2, 7)]
                    for s4 in range(4):
                        c0, c1 = proj_chunks("k", mm, s4)
                        queue_at(*kd[3 * s4], fn=c0)
                        queue_at(*kd[3 * s4 + 1], fn=c1)
                        queue_at(*kd[3 * s4 + 2],
                                 fn=lambda s4=s4: fold(
                                     KT8[mm], s4 * 512, (s4 + 1) * 512))
                    vd = [(u2, 6), (u2, 8), (u2, 8), (u2, 10),
                          (u2, 10), (u2, 12), (u2, 12), (u2, 14)]
                    for s4 in range(4):
                        c0, c1 = proj_chunks("v", mm, s4)
                        queue_at(*vd[2 * s4], fn=c0)
                        queue_at(*vd[2 * s4 + 1], fn=c1)
                    h0d = [(u2, 9), (u2, 11), (u2, 13), (u2, 15)]
                    h1d = [(u2 + 1, 1), (u2 + 1, 3),
                           (u2 + 1, 5), (u2 + 1, 7)]
                    for s4 in range(4):
                        for i, t in enumerate(vpath_quarter(2 * mm, s4)):
                            du, dk = h0d[s4]
                            queue_at(du, dk + i, t)
                        for i, t in enumerate(vpath_quarter(2 * mm + 1, s4)):
                            du, dk = h1d[s4]
                            queue_at(du, min(dk + i, 15), t)
                    if mm == 2:
                        queue_at(u2, 12, load_wo)

                def queue_qhalf1(m):
                    # qb=1 Q halves, deferred into the qb=1 phase (first
                    # needed by unit 6 + 2m; chain ends ~8 slots early so
                    # the evac+fold latency never reaches the scores)
                    d = 5 + 2 * m
                    for s4 in (2, 3):
                        c0, c1 = proj_chunks("q", m, s4)
                        queue_at(d, 4 * (s4 - 2), c0)
                        queue_at(d, 4 * (s4 - 2) + 2, c1)
                    queue_at(d, 7, lambda: fold(QT8[m], QB, S))

                def av_norm_tasks(h, qb, ptb, last):
                    """unit (h, qb): AV in two independent aux slots (one
                    per 512-q half) + late normalize; hh=1 units append a
                    per-half transpose so the tail only waits on half a
                    unit.  `last` queues the qb out-projections behind the
                    transposes."""
                    m, hh = h // 2, h % 2
                    po = hh * 64
                    ptv = ptb[:].rearrange("p (t c) -> p t c", t=KT16)
                    v1v = v18[h][:].rearrange("p (t c) -> p t c", t=KT16)
                    r1v = r8[h][:].rearrange("p (t c) -> p t c", t=KT16)
                    st = {}
                    tasks = []

                    def mk_av(pa, jlo):
                        def run():
                            if jlo == 0:
                                st[pa] = aux_tile()
                            at = st[pa]
                            for j in range(jlo, jlo + 4):
                                for qt in range(4 * pa, 4 * pa + 4):
                                    lh = ptv[:, 2 * j:2 * j + 2,
                                             qt * 128:(qt + 1) * 128]
                                    oc = at[:, (qt % 4) * 128:
                                            (qt % 4) * 128 + 65]
                                    nc.tensor.matmul(
                                        oc, lh, v1v[:, 2 * j:2 * j + 2, :],
                                        start=(j == 0 and qt % 4 == 0),
                                        stop=False, perf_mode=DR,
                                        skip_group_check=True,
                                    )
                                    nc.tensor.matmul(
                                        oc, lh, r1v[:, 2 * j:2 * j + 2, :],
                                        start=False,
                                        stop=(j == 7 and qt % 4 == 3),
                                        perf_mode=DR,
                                        skip_group_check=True,
                                    )
                        return run

                    def mk_norm(pa):
                        def run():
                            at = st[pa]
                            rc = NW.tile([128, 4], F32, tag="rc", name="rc")
                            atv = at[:].rearrange("p (t c) -> p t c", t=4)
                            nc.vector.reciprocal(rc[:], atv[:, :, 64:65])
                            asv = attn_sb[(qb, m)][:].rearrange(
                                "p (t c) -> p t c", t=8)
                            nc.vector.tensor_tensor(
                                asv[:, 4 * pa:4 * pa + 4, po:po + 64],
                                atv[:, :, 0:64],
                                rc[:].unsqueeze(2).broadcast_to([128, 4, 64]),
                                ALU.mult)
                        return run

                    def mk_trans(pa):
                        def run():
                            if last and qb == 1:
                                # tail-critical: PE transpose via identity +
                                # engine evac skips the ~2.6us DMA launch +
                                # completion-sem latency of the xbar path
                                tp = SP.tile([128, 1024], BF16, tag="aux",
                                             bufs=2, name="aux")
                                for i in range(4):
                                    cs = slice(pa * 512 + i * 128,
                                               pa * 512 + (i + 1) * 128)
                                    nc.tensor.matmul(
                                        tp[:, i * 128:(i + 1) * 128],
                                        attn_sb[(qb, m)][:, cs],
                                        ident[:], is_transpose=True,
                                        skip_group_check=True)
                                dst = attnT[m][:, qb * QB + pa * 512:
                                               qb * QB + (pa + 1) * 512]
                                if pa == 0:
                                    nc.scalar.activation(dst, tp[:, 0:512],
                                                         AF.Copy)
                                else:
                                    nc.vector.tensor_copy(dst, tp[:, 0:512])
                            else:
                                nc.sync.dma_start_transpose(
                                    attnT[m][:, qb * QB + pa * 512:
                                             qb * QB + (pa + 1) * 512]
                                    .rearrange("p (t c) -> p t c", t=4),
                                    attn_sb[(qb, m)][:, pa * 512:(pa + 1) * 512])
                            if last:
                                tl = [8 * qb + 4 * pa + i for i in range(4)]
                                if qb == 0:
                                    # ~1.3 out tiles per qb=1 unit keeps
                                    # PE under the exp cadence there
                                    for i, t in enumerate(tl):
                                        s = 8 * (2 * i + pa) + 8
                                        queue_at(6 + s // 16, (s % 16) + 2,
                                                 out_proj_task(t, "dve"))
                                elif pa == 0:
                                    # t8-11 only need the pa0 transpose:
                                    # overlap them with unit 11's back half
                                    for i, t in enumerate(tl):
                                        queue_at(11, 8 + 2 * i - (i == 3),
                                                 out_proj_task(
                                                     t, ["act", "dve"][i % 2]))
                                else:
                                    for t, eng in zip(tl, ["act", "dve",
                                                           "act", "dve"]):
                                        queue_at(12, 0,
                                                 out_proj_task(t, eng))
                        return run

                    # jlo=0 passes only read the previous unit's kc 0..7
                    # exp tiles (long done); the jlo=4 passes need kc 14/15
                    # which are still in flight at slot 0 -- run both jlo=0
                    # passes first so PE never head-blocks, and push the
                    # norms late so the reciprocal's AV wait is satisfied
                    # by emission time.
                    tasks.append(mk_av(0, 0))
                    tasks.append(mk_av(1, 0))
                    tasks.append(mk_av(0, 4))
                    tasks.append(mk_av(1, 4))
                    tasks.append(lambda: None)
                    tasks.append(mk_norm(0))
                    if hh == 1:
                        tasks.append(mk_trans(0))
                    tasks.append(mk_norm(1))
                    if hh == 1:
                        tasks.append(mk_trans(1))
                    return tasks

                def pop_task(kc=1, startup=False):
                    # slots 0-3 of each unit: no pops at startup (let the
                    # score/exp pipeline fill), cold-only otherwise; hot
                    # (AV/norm/trans) from slot 4 so AV's exp inputs are
                    # long done by emission and PE never head-blocks
                    if kc < 4:
                        if not startup and cold:
                            heapq.heappop(cold)[3]()
                        return
                    if hot:
                        hot.popleft()()
                    elif cold:
                        heapq.heappop(cold)[3]()
                        if cold and len(cold) > 12 and kc % 2 == 1:
                            heapq.heappop(cold)[3]()

                # ---------- m = 0 startup: the s4=0 K chunk (gated only on
                # the first quarter of x) and both qb=0 Q chunks run inline
                # on the aux ring, with per-s4 folds, so unit 0's scores
                # pipeline from the first tile.  Everything else rides the
                # task queue. ---------------------------------------------
                for p, s4 in (("k", 0), ("q", 0), ("q", 1)):
                    c0, c1 = proj_chunks(p, 0, s4)
                    c0(); c1()
                fold(KT8[0], 0, 512)
                fold(QT8[0], 0, QB)
                # K s4-1 only gates kc4+ (kc0-3 run non-DR off the raw
                # region): ride the cold queue so the first exp tile is
                # one projection chunk earlier
                c0, c1 = proj_chunks("k", 0, 1)
                queue_at(0, 0, c0)
                queue_at(0, 2, c1)
                queue_at(0, 3, lambda: fold(KT8[0], 512, QB))
                queue_at(0, 4, load_w_rest("q"))
                queue_at(0, 6, load_w_rest("k"))
                kd0 = [(0, 4), (0, 6), (0, 7), (0, 8), (0, 10), (0, 11)]
                for s4 in (2, 3):
                    c0, c1 = proj_chunks("k", 0, s4)
                    i = 3 * (s4 - 2)
                    queue_at(*kd0[i], fn=c0)
                    queue_at(*kd0[i + 1], fn=c1)
                    queue_at(*kd0[i + 2],
                             fn=lambda s4=s4: fold(
                                 KT8[0], s4 * 512, (s4 + 1) * 512))
                vd0 = [(0, 4), (0, 6), (0, 6), (0, 8),
                       (0, 8), (0, 10), (0, 10), (0, 12)]
                for s4 in range(4):
                    c0, c1 = proj_chunks("v", 0, s4)
                    queue_at(*vd0[2 * s4], fn=c0)
                    queue_at(*vd0[2 * s4 + 1], fn=c1)
                h0d0 = [(0, 7), (0, 9), (0, 11), (0, 13)]
                h1d0 = [(0, 11), (0, 13), (0, 15), (1, 1)]
                for s4 in range(4):
                    for i, t in enumerate(vpath_quarter(0, s4)):
                        du, dk = h0d0[s4]
                        queue_at(du, min(dk + i, 15), t)
                    for i, t in enumerate(vpath_quarter(1, s4)):
                        du, dk = h1d0[s4]
                        queue_at(du, dk + i if dk + i <= 15 else 15, t)

                # ---------- pipelined attention units ---------------------
                # qb-major order: all qb=0 attention completes by unit 5,
                # so out tiles 0-7 overlap the entire qb=1 phase.
                units = [(qb, m, hh)
                         for qb in range(2) for m in range(3) for hh in range(2)]
                for ui, (qb, m, hh) in enumerate(units):
                    h = 2 * m + hh
                    po = hh * 64
                    if qb == 0 and hh == 0 and m + 1 < 3:
                        queue_mchunk(m + 1)
                    if qb == 0 and hh == 1:
                        queue_qhalf1(m)

                    ptb = Pt[ui % 2]
                    ktv = KT8[m][po:po + 32, :].rearrange(
                        "p (t c) -> p t c", t=2)
                    qtv = QT8[m][po:po + 32, :].rearrange(
                        "p (t c) -> p t c", t=2)
                    unit_tasks = av_norm_tasks(
                        h, qb, ptb, last=(m == 2 and hh == 1))
                    base_w = EXP_W0 if qb == 0 else EXP_W1
                    for kc in range(KT16):
                        fence(ui, kc)
                        eng = exp_engine(base_w)
                        sps = sc_tile()
                        for jh in range(2):
                            qcols = slice(qb * QB + jh * 512,
                                          qb * QB + (jh + 1) * 512)
                            if ui == 0 and kc < 4:
                                # startup: non-DoubleRow off the unfolded
                                # region, so the fold DMAs stay off the
                                # critical path to the first exp tile
                                nc.tensor.matmul(
                                    sps[:, jh * 512:(jh + 1) * 512],
                                    KT8[m][po:po + 64,
                                           kc * 128:(kc + 1) * 128],
                                    QT8[m][po:po + 64, qcols],
                                    start=True, stop=True,
                                )
                            else:
                                nc.tensor.matmul(
                                    sps[:, jh * 512:(jh + 1) * 512],
                                    ktv[:, :, kc * 128:(kc + 1) * 128],
                                    qtv[:, :, qcols],
                                    start=True, stop=True, perf_mode=DR,
                                )
                        pcols = slice(kc * QB, (kc + 1) * QB)
                        if eng == "act":
                            nc.scalar.activation(
                                ptb[:, pcols], sps[:], AF.Exp, scale=SCALE)
                        else:
                            nc.vector.tensor_scalar(
                                ptb[:, pcols].bitcast(U8), sps[:],
                                SCH_A, SCH_B, ALU.mult, ALU.add)
                        pop_task(kc=kc, startup=(ui == 0))
                        if ui == len(units) - 1 and kc in (11, 13):
                            # the jlo=0 AV passes only read this unit's
                            # kc 0-7 exp tiles: run them inside the kc
                            # loop so the drain chain shortens
                            unit_tasks.pop(0)()

                    hot.extend(unit_tasks)

                # ---------- drain (tail out-projections ride the queues) --
                while hot or cold:
                    if hot:
                        hot.popleft()()
                    else:
                        heapq.heappop(cold)[3]()

    split_waits(nc)
    return nc


_NC_CACHE = None


def _get_nc():
    global _NC_CACHE
    if _NC_CACHE is None:
        _NC_CACHE = build_nc()
    return _NC_CACHE


def make_in_maps(x, QW_w, QW_b, KW_w, KW_b, VW_w, VW_b, OW_w, OW_b):
    f32 = lambda a: np.ascontiguousarray(np.asarray(a), dtype=np.float32)
    bf = lambda a: np.ascontiguousarray(
        np.asarray(np.asarray(a), dtype=np.float32)).astype(ml_dtypes.bfloat16)
    in_maps = []
    for c in range(N_CORES):
        b, hh = c // 2, c % 2
        sl = slice(hh * FS, (hh + 1) * FS)
        in_maps.append({
            "xt": bf(np.asarray(x[b]).T),
            "wqt": bf(np.asarray(QW_w)[sl, :].T),
            "wkt": bf(np.asarray(KW_w)[sl, :].T),
            "wvt": bf(np.asarray(VW_w)[sl, :].T),
            "wot": bf(np.asarray(OW_w)[:, sl].T),
            "bq": f32(QW_b[sl]),
            "bk": f32(KW_b[sl]),
            "ident": np.eye(128, dtype=ml_dtypes.bfloat16),
        })
    return in_maps


def kernel(x, QW_w, QW_b, KW_w, KW_b, VW_w, VW_b, OW_w, OW_b):
    nc = _get_nc()
    in_maps = make_in_maps(x, QW_w, QW_b, KW_w, KW_b, VW_w, VW_b, OW_w, OW_b)
    res = run_bass_kernel_spmd(nc, in_maps, list(range(N_CORES)))

    out = np.zeros((B, S, D), dtype=np.float32)
    OW_w = np.asarray(OW_w, dtype=np.float32)
    OW_b = np.asarray(OW_b, dtype=np.float32)
    VW_b = np.asarray(VW_b, dtype=np.float32)
    for c in range(N_CORES):
        b = c // 2
        out[b] += res.results[c]["out_part"].astype(np.float32)
    for b in range(B):
        # OW bias + V-bias routed through the output projection
        out[b] += OW_b + OW_w @ VW_b
    return out
